# revision 2
# baseline (speedup 1.0000x reference)
"""Trainium2 Bass kernel for nn_Logic_53068615909594 — v6.

Math: the reference collapses to a per-column-pair bilinear polynomial

    Y[s, 2q+o] = a0[k] + a1[k]*x0 + a2[k]*x1 + a3[k]*x0*x1,  k = 2q+o

computed as (Horner split, only fast-mode ops):

    Ye = pe + x1*re,  pe = a1e*x0 + a0e,  re = a3e*x0 + a2e
    Yo = po + x0*ro,  po = a2o*x1 + a0o,  ro = a3o*x1 + a1o

Engine split: ACT does pe/po (+ ro on the bookend blocks); DVE does the
remaining affines as tensor_scalar (4x mode) and the four tensor_tensor
ops (2x mode) per chunk. ACT producers run one chunk ahead of DVE.
No GpSimd compute: its software vector ops are slow and their SBUF
traffic degrades DVE throughput (measured v5).

Layout: column pairs on SBUF partitions (host transpose); fp16 X/Y HBM
traffic; per-partition f32 coefficient scalars. Loads on SP's hardware
DGE queue; stores on GpSimd's software DGE queue.

Sharding: 2 row-groups x 4 col-groups -> per core 4096 rows x 512 pairs
= 4 partition blocks. First and last block are split into two 2048-row
chunks to shorten pipeline ramp and tail; middle blocks run full 4096.
"""

import os
import numpy as np

N_SLOW = 8192
NUM_IN = 4096
N_CORES = 8
RG, CG = 2, 4                          # row-groups x col-groups
ROWS = N_SLOW // RG                    # 4096 rows per core
PAIRS = (NUM_IN // 2) // CG            # 512 pairs per core
PB = 128                               # partition block
NBLK = PAIRS // PB                     # 4 partition blocks per core

# chunk list: (block j, row r0, row r1, ro_on_act)
CHUNKS = [(0, 0, ROWS // 2, True), (0, ROWS // 2, ROWS, True),
          (1, 0, ROWS, False), (2, 0, ROWS, False),
          (3, 0, ROWS // 2, True), (3, ROWS // 2, ROWS, True)]

_BUILD_CACHE = {}

# test.py introspection: last BassKernelResults (set when KERNEL_TRACE=1)
LAST_RESULTS = None


def _build_bass():
    import concourse.bass as bass
    import concourse.tile as tile
    from concourse import bacc, mybir

    f16 = mybir.dt.float16
    f32 = mybir.dt.float32
    mult = mybir.AluOpType.mult
    add = mybir.AluOpType.add
    ident = mybir.ActivationFunctionType.Identity

    nc = bacc.Bacc("TRN2", target_bir_lowering=False, debug=False,
                   num_devices=N_CORES)
    XE_d = nc.dram_tensor("XE", [PAIRS, ROWS], f16, kind="ExternalInput")
    XO_d = nc.dram_tensor("XO", [PAIRS, ROWS], f16, kind="ExternalInput")
    CF_d = nc.dram_tensor("CF", [PB, NBLK * 8], f32, kind="ExternalInput")
    YE_d = nc.dram_tensor("YE", [PAIRS, ROWS], f16, kind="ExternalOutput")
    YO_d = nc.dram_tensor("YO", [PAIRS, ROWS], f16, kind="ExternalOutput")

    with tile.TileContext(nc) as tc:
        with tc.tile_pool(name="coef", bufs=1) as cp, \
             tc.tile_pool(name="xin", bufs=3) as xp, \
             tc.tile_pool(name="pre", bufs=2) as pp, \
             tc.tile_pool(name="tmp", bufs=2) as tp, \
             tc.tile_pool(name="yout", bufs=2) as yp:
            CF = cp.tile([PB, NBLK * 8], f32)
            nc.sync.dma_start(CF[:], CF_d.ap())
            cf = CF[:]

            XE_ap = XE_d.ap()
            XO_ap = XO_d.ap()
            YE_ap = YE_d.ap()
            YO_ap = YO_d.ap()

            def coefs(j):
                return [cf[:, j * 8 + k:j * 8 + k + 1] for k in range(8)]

            def load(c):
                j, r0, r1, _ = CHUNKS[c]
                W = r1 - r0
                xe = xp.tile([PB, ROWS], f16, tag="xe")
                xo = xp.tile([PB, ROWS], f16, tag="xo")
                ps = slice(j * PB, (j + 1) * PB)
                nc.sync.dma_start(xe[:][:, 0:W], XE_ap[ps, r0:r1])
                nc.sync.dma_start(xo[:][:, 0:W], XO_ap[ps, r0:r1])
                return xe, xo

            def produce(c, xe, xo):
                # one-chunk-ahead affine producers on ACT
                j, r0, r1, ro_act = CHUNKS[c]
                W = r1 - r0
                a0e, a1e, a2e, a3e, a0o, a1o, a2o, a3o = coefs(j)
                pe = pp.tile([PB, ROWS], f16, tag="pe")
                nc.scalar.activation(pe[:][:, 0:W], xe[:][:, 0:W], ident,
                                     bias=a0e, scale=a1e)
                po = pp.tile([PB, ROWS], f16, tag="po")
                nc.scalar.activation(po[:][:, 0:W], xo[:][:, 0:W], ident,
                                     bias=a0o, scale=a2o)
                ro = None
                if ro_act:
                    ro = pp.tile([PB, ROWS], f16, tag="ro")
                    nc.scalar.activation(ro[:][:, 0:W], xo[:][:, 0:W], ident,
                                         bias=a1o, scale=a3o)
                return pe, po, ro

            NC = len(CHUNKS)
            xin = {0: load(0)}
            pre = {0: produce(0, *xin[0])}
            for c in range(NC):
                j, r0, r1, ro_act = CHUNKS[c]
                W = r1 - r0
                if c + 1 < NC:
                    xin[c + 1] = load(c + 1)
                    pre[c + 1] = produce(c + 1, *xin[c + 1])
                xe_t, xo_t = xin.pop(c)
                pe_t, po_t, ro_t = pre.pop(c)
                xe, xo = xe_t[:][:, 0:W], xo_t[:][:, 0:W]
                pe, po = pe_t[:][:, 0:W], po_t[:][:, 0:W]
                a0e, a1e, a2e, a3e, a0o, a1o, a2o, a3o = coefs(j)
                ps = slice(j * PB, (j + 1) * PB)

                # even outputs: Ye = pe + x1*(a3e*x0 + a2e)
                re_t = tp.tile([PB, ROWS], f16, tag="re")
                re = re_t[:][:, 0:W]
                nc.vector.tensor_scalar(re, xe, a3e, a2e, mult, add)
                se_t = tp.tile([PB, ROWS], f16, tag="se")
                se = se_t[:][:, 0:W]
                nc.vector.tensor_mul(se, xo, re)
                ye_t = yp.tile([PB, ROWS], f16, tag="ye")
                ye = ye_t[:][:, 0:W]
                nc.vector.tensor_add(ye, pe, se)

                # odd outputs: Yo = po + x0*(a3o*x1 + a1o)
                if ro_t is None:
                    ro_t = tp.tile([PB, ROWS], f16, tag="rod")
                    ro = ro_t[:][:, 0:W]
                    nc.vector.tensor_scalar(ro, xo, a3o, a1o, mult, add)
                else:
                    ro = ro_t[:][:, 0:W]
                so_t = tp.tile([PB, ROWS], f16, tag="so")
                so = so_t[:][:, 0:W]
                nc.vector.tensor_mul(so, xe, ro)
                yo_t = yp.tile([PB, ROWS], f16, tag="yo")
                yo = yo_t[:][:, 0:W]
                nc.vector.tensor_add(yo, po, so)

                nc.gpsimd.dma_start(YE_ap[ps, r0:r1], ye)
                nc.gpsimd.dma_start(YO_ap[ps, r0:r1], yo)
    nc.compile()
    return nc


def _coeffs(P):
    """a0..a3 (each (NUM_IN,)) of the direct bilinear form, from P (4, nOut)."""
    P = np.asarray(P, dtype=np.float64)
    H = np.ones((1, 1))
    for _ in range(2):
        H = np.block([[H, -H], [H, H]])
    B = H @ P                                        # (4, nOut)
    k = np.arange(4)
    s0 = (2.0 * ((k >> 0) & 1) - 1.0)
    s1 = (2.0 * ((k >> 1) & 1) - 1.0)
    a0 = 0.25 * B.sum(axis=0)
    a1 = 0.25 * (s0[:, None] * B).sum(axis=0)
    a2 = 0.25 * (s1[:, None] * B).sum(axis=0)
    a3 = 0.25 * ((s0 * s1)[:, None] * B).sum(axis=0)
    return a0, a1, a2, a3


def _pack_cf(P):
    """Per col-group (PB, NBLK*8) f32 coefficient blocks: partition p,
    slot 8j+k holds coef k (a0e,a1e,a2e,a3e,a0o,a1o,a2o,a3o) for pair
    q = cg*PAIRS + j*PB + p."""
    a0, a1, a2, a3 = _coeffs(P)
    A = np.stack([a0[0::2], a1[0::2], a2[0::2], a3[0::2],
                  a0[1::2], a1[1::2], a2[1::2], a3[1::2]], axis=1)  # (2048,8)
    out = []
    for cg in range(CG):
        Acg = A[cg * PAIRS:(cg + 1) * PAIRS]
        cfp = Acg.reshape(NBLK, PB, 8).transpose(1, 0, 2).reshape(PB, NBLK * 8)
        out.append(np.ascontiguousarray(cfp.astype(np.float32)))
    return out


def _install_ntff_shim():
    """The image's antenv package lacks axon_hooks; recreate it and register
    the ctypes NTFF profile hook so trace=True yields exec_time_ns. Also
    neuter upload_artifacts (no bucket creds in this container)."""
    import sys
    import types
    try:
        from antenv.axon_hooks import get_axon_ntff_profile_hook  # noqa: F401
    except ImportError:
        import antenv
        m = types.ModuleType("antenv.axon_hooks")
        holder = {"hook": None}
        m.set_axon_ntff_profile_hook = lambda h: holder.__setitem__("hook", h)
        m.get_axon_ntff_profile_hook = lambda: holder["hook"]
        sys.modules["antenv.axon_hooks"] = m
        antenv.axon_hooks = m
    from antenv.axon_hooks import (  # noqa: F811
        get_axon_ntff_profile_hook, set_axon_ntff_profile_hook,
    )
    if get_axon_ntff_profile_hook() is None:
        from trn_agent_boot.trn_boot import _ntff_profile_via_ctypes
        set_axon_ntff_profile_hook(
            _ntff_profile_via_ctypes("/opt/axon/libaxon_pjrt.so"))
    from concourse import bass_utils
    bass_utils.upload_artifacts = lambda tmpdir: f"local:{tmpdir}"


def kernel(X, P):
    global LAST_RESULTS
    from concourse import bass_utils

    X = np.asarray(X)
    CFs = _pack_cf(P)

    if "nc" not in _BUILD_CACHE:
        _BUILD_CACHE["nc"] = _build_bass()
    nc = _BUILD_CACHE["nc"]

    in_maps = []
    for i in range(N_CORES):
        rg, cg = divmod(i, CG)
        rs = slice(rg * ROWS, (rg + 1) * ROWS)
        cs = slice(cg * 2 * PAIRS, (cg + 1) * 2 * PAIRS)
        Xi = X[rs, cs]
        XEi = np.ascontiguousarray(Xi[:, 0::2].T.astype(np.float16))
        XOi = np.ascontiguousarray(Xi[:, 1::2].T.astype(np.float16))
        in_maps.append({"XE": XEi, "XO": XOi, "CF": CFs[cg]})

    trace = os.environ.get("KERNEL_TRACE", "0") == "1"
    if trace:
        _install_ntff_shim()
    res = bass_utils.run_bass_kernel_spmd(
        nc, in_maps, core_ids=list(range(N_CORES)), trace=trace,
        tmpdir=os.environ.get("KERNEL_TRACE_DIR") or None,
    )
    LAST_RESULTS = res
    Y = np.empty((N_SLOW, NUM_IN), dtype=np.float32)
    for i in range(N_CORES):
        rg, cg = divmod(i, CG)
        rs = slice(rg * ROWS, (rg + 1) * ROWS)
        c0 = cg * 2 * PAIRS
        Y[rs, c0 + 0:c0 + 2 * PAIRS:2] = res.results[i]["YE"].T.astype(
            np.float32)
        Y[rs, c0 + 1:c0 + 2 * PAIRS:2] = res.results[i]["YO"].T.astype(
            np.float32)
    return Y
